# revision 7
# baseline (speedup 1.0000x reference)
"""GriffinBlock1D Trainium2 Bass kernel (v2 — bf16).

Sharding: 8 cores = (batch b, T-half). GLRU (matmuls + scan over full T) is
replicated within each batch pair; attention + FFN computed per 512-token
half (16-token halo selected data-driven so the SPMD program is identical).

v2 changes vs v1:
- All matmul operands bf16 (fp32r runs 2 passes/row on TRN2 HW; bf16 is 1).
- W_state folded into W_in on host: cand = x @ (W_state @ W_in[D:]).T.
- LayerNorm stats & softmax denominators produced in broadcast form
  (ones[128,K] stationary -> all 128 output partitions equal), so no
  outer-product G tiles, no gpsimd partition_broadcast, no DVE reciprocal:
  1/x computed as Exp(-Ln(x)) on the scalar engine, staying in the
  natural_log_exp activation table.
- Activation-table selection patched so Ln/Exp share one table (avoids
  per-LN ACT_TABLE_LOAD thrash).
- Attention banded scores in 224/224/64 q-blocks with 128-row k-pieces.
- bf16 DMA halves input bandwidth; weights prefetched eagerly.
"""

import numpy as np
import ml_dtypes

import concourse.bass as bass
import concourse.mybir as mybir
import concourse.tile as tile
from concourse import bacc
from concourse.bass_utils import run_bass_kernel_spmd

F32 = mybir.dt.float32
BF16 = mybir.dt.bfloat16
AF = mybir.ActivationFunctionType
ALU = mybir.AluOpType

B, T, D, H, WIN, FFD = 4, 1024, 512, 4, 16, 2048
DH = D // H          # 128
TL = T // 2          # 512 tokens per core
WT = TL + 2 * WIN    # 544-token window (with halo)
EPS = 1e-5
SCL = 1.0 / np.sqrt(DH)
NCORES = 8

# attention geometry: q-blocks in x1-column space [16, 528); k-pieces of
# <=128 rows; (q0, qn, [(pst, plen), ...], mask col offset)
QBLOCKS = [
    (16, 224, [(0, 128), (128, 128)], 0),
    (240, 224, [(224, 128), (352, 128)], 448),
    (464, 64, [(448, 96)], 896),
]
VSTARTS = [(0, 128), (128, 128), (224, 128), (352, 128), (448, 96)]
MSKW = 960

_CACHE = {}


def _patch_act_tables():
    """Make Ln/Exp resolve to the combined natural_log_exp table so the
    scalar engine does not reload its function table between Ln and Exp."""
    if _CACHE.get("act_patched"):
        return
    from concourse.hw_specs import get_activation_tables as orig

    def patched(arch):
        t = dict(orig(arch))
        keep = "natural_log_exp_and_others"
        if keep in t:
            for name in list(t):
                if name != keep:
                    t[name] = t[name] - {AF.Ln, AF.Exp}
        return t

    bacc.get_activation_tables = patched
    _CACHE["act_patched"] = True


def _build_nc():
    _patch_act_tables()
    nc = bacc.Bacc("TRN2", target_bir_lowering=False, debug=False)

    di = lambda n, s, dt=BF16: nc.dram_tensor(n, s, dt, kind="ExternalInput")
    xt_d = di("xt", [D, T])
    wu_d = di("wu", [D, D])            # W_in[:D].T
    wc_d = di("wc", [D, D])            # (W_state @ W_in[D:]).T
    wq_d = di("wq", [D, D])
    wk_d = di("wk", [D, D])
    wv_d = di("wv", [D, D])
    wp_d = di("wp", [D, D])
    w1_d = di("w1", [D, FFD])
    w2_d = di("w2", [FFD, D])
    lncol_d = di("lncol", [128, 4, 8], F32)   # [p, et, 2*ln + (g|b)]
    b1_d = di("b1c", [128, FFD // 128], F32)
    b2_d = di("b2c", [128, D // 128], F32)
    cw_d = di("cw", [128, 2], F32)
    msk_d = di("msk", [128, MSKW], BF16)
    out_d = nc.dram_tensor("outp", [4, 128, TL], F32, kind="ExternalOutput")

    rr = lambda ap: ap.rearrange("(a p) e -> p a e", p=128)

    with nc.allow_low_precision(reason="bf16 activations; LN renormalizes"), \
         tile.TileContext(nc) as tc:
        # PSUM tags (8 banks total): mm(2) + cd(2) + sc(2) + s12(2)
        with tc.tile_pool(name="cp", bufs=1) as cp, \
             tc.tile_pool(name="sq", bufs=2) as sqp, \
             tc.tile_pool(name="lnt", bufs=2) as lnp, \
             tc.tile_pool(name="ep", bufs=3) as ep, \
             tc.tile_pool(name="pp", bufs=2, space="PSUM") as pm:
            pb = ps2 = pa = pm

            # ---------------- inputs / constants ----------------
            wu_sb = cp.tile([128, 4, D], BF16, tag="wu")
            nc.sync.dma_start(wu_sb, rr(wu_d[:, :]))
            xt_sb = cp.tile([128, 4, T], BF16, tag="xt")
            nc.sync.dma_start(xt_sb[:, :, 0:TL], xt_d[:, 0:TL].rearrange("(a p) t -> p a t", p=128))
            wc_sb = cp.tile([128, 4, D], BF16, tag="wc")
            nc.sync.dma_start(wc_sb, rr(wc_d[:, :]))
            nc.sync.dma_start(xt_sb[:, :, TL:T], xt_d[:, TL:T].rearrange("(a p) t -> p a t", p=128))
            wqk_sb = cp.tile([128, 8, D], BF16, tag="wqk")
            nc.sync.dma_start(wqk_sb[:, 0:4, :], rr(wq_d[:, :]))
            nc.sync.dma_start(wqk_sb[:, 4:8, :], rr(wk_d[:, :]))
            wv_sb = cp.tile([128, 4, D], BF16, tag="wv")
            nc.sync.dma_start(wv_sb, rr(wv_d[:, :]))
            wp_sb = cp.tile([128, 4, D], BF16, tag="wp")
            nc.sync.dma_start(wp_sb, rr(wp_d[:, :]))
            msk_sb = cp.tile([128, MSKW], BF16, tag="msk")
            nc.sync.dma_start(msk_sb, msk_d[:, :])
            lncol_sb = cp.tile([128, 4, 8], F32, tag="lncol")
            nc.sync.dma_start(lncol_sb, lncol_d[:, :, :])
            b1_sb = cp.tile([128, FFD // 128], F32, tag="b1")
            nc.sync.dma_start(b1_sb, b1_d[:, :])
            b2_sb = cp.tile([128, D // 128], F32, tag="b2")
            nc.sync.dma_start(b2_sb, b2_d[:, :])
            cw_sb = cp.tile([128, 2], F32, tag="cw")
            nc.sync.dma_start(cw_sb, cw_d[:, :])
            w1_sb = cp.tile([128, 4, FFD], BF16, tag="w1")
            nc.sync.dma_start(w1_sb, rr(w1_d[:, :]))
            w2_sb = cp.tile([128, 16, D], BF16, tag="w2")
            nc.sync.dma_start(w2_sb, rr(w2_d[:, :]))

            ones_sb = cp.tile([128, 128], BF16, tag="ones")
            nc.vector.memset(ones_sb, 1.0)
            epsc = cp.tile([128, 1], F32, tag="epsc")
            nc.vector.memset(epsc, EPS)

            # ---------------- GLRU: u / cand matmuls + scan (et-pipelined) ----------------
            u_sb = cp.tile([128, 4, T], BF16, tag="u")
            w_sb = cp.tile([128, 4, T], F32, tag="w")
            y_sb = cp.tile([128, 4, T], BF16, tag="y")
            ywin = cp.tile([128, 4, WT], BF16, tag="ywin")
            for et in range(4):
                for nch in range(2):
                    tsl = slice(nch * 512, (nch + 1) * 512)
                    g1 = pm.tile([128, 512], F32, tag="mm")
                    for kk in range(4):
                        nc.tensor.matmul(
                            g1, wu_sb[:, kk, et * 128:(et + 1) * 128],
                            xt_sb[:, kk, tsl], start=kk == 0, stop=kk == 3)
                    nc.scalar.activation(u_sb[:, et, tsl], g1, AF.Sigmoid)
                    nc.scalar.activation(w_sb[:, et, tsl], g1, AF.Sigmoid,
                                         scale=-1.0)
                    cd = pb.tile([128, 512], F32, tag="cd")
                    for kk in range(4):
                        nc.tensor.matmul(
                            cd, wc_sb[:, kk, et * 128:(et + 1) * 128],
                            xt_sb[:, kk, tsl], start=kk == 0, stop=kk == 3)
                    # w = sigmoid(-gv1) * cand
                    nc.vector.tensor_mul(w_sb[:, et, tsl], w_sb[:, et, tsl], cd)
                nc.vector.tensor_tensor_scan(y_sb[:, et, :], u_sb[:, et, :],
                                             w_sb[:, et, :], 0.0,
                                             ALU.mult, ALU.add)
                yw = ywin[:, et, :]
                nc.vector.tensor_scalar(yw[:, WIN:WIN + TL], y_sb[:, et, 0:TL],
                                        cw_sb[:, 0:1], None, ALU.mult)
                nc.vector.scalar_tensor_tensor(
                    yw[:, WIN:WIN + TL], y_sb[:, et, TL:T], cw_sb[:, 1:2],
                    yw[:, WIN:WIN + TL], ALU.mult, ALU.add)
                nc.vector.tensor_scalar(yw[:, 0:WIN], y_sb[:, et, TL - WIN:TL],
                                        cw_sb[:, 1:2], None, ALU.mult)
                nc.vector.tensor_scalar(yw[:, WIN + TL:WT],
                                        y_sb[:, et, TL:TL + WIN],
                                        cw_sb[:, 0:1], None, ALU.mult)

            # ---------------- LayerNorm (broadcast-form stats) ----------------
            def layer_norm(xin, ln_idx, out_get, ncols, mm_in=None):
                mm_in = mm_in or xin
                for c0 in range(0, ncols, 512):
                    cn = min(512, ncols - c0)
                    cs = slice(c0, c0 + cn)
                    s1b = pa.tile([128, cn], F32, tag="s12")
                    s2b = pa.tile([128, cn], F32, tag="s12")
                    for et in range(4):
                        sq = sqp.tile([128, cn], BF16, tag="sq")
                        nc.scalar.activation(sq, mm_in(et)[:, cs], AF.Square)
                        nc.tensor.matmul(s1b, ones_sb, mm_in(et)[:, cs],
                                         start=et == 0, stop=et == 3)
                        nc.tensor.matmul(s2b, ones_sb, sq,
                                         start=et == 0, stop=et == 3)
                    mb = lnp.tile([128, cn], F32, tag="mb")
                    nc.scalar.activation(mb, s1b, AF.Copy, scale=1.0 / D)
                    m2b = lnp.tile([128, cn], F32, tag="m2b")
                    nc.scalar.activation(m2b, s1b, AF.Square, scale=1.0 / D)
                    vb = lnp.tile([128, cn], F32, tag="vb")
                    nc.vector.scalar_tensor_tensor(
                        vb, s2b, 1.0 / D, m2b, ALU.mult, ALU.subtract)
                    lnv = lnp.tile([128, cn], F32, tag="lnv")
                    nc.scalar.activation(lnv, vb, AF.Ln, bias=epsc)
                    rb = lnp.tile([128, cn], F32, tag="rb")
                    nc.scalar.activation(rb, lnv, AF.Exp, scale=-0.5)
                    for et in range(4):
                        o = out_get(et)[:, cs]
                        t1 = sqp.tile([128, cn], BF16, tag="t1")
                        nc.vector.tensor_sub(t1, xin(et)[:, cs], mb)
                        nc.vector.tensor_mul(t1, t1, rb)
                        nc.vector.tensor_scalar(
                            o, t1, lncol_sb[:, et, 2 * ln_idx:2 * ln_idx + 1],
                            lncol_sb[:, et, 2 * ln_idx + 1:2 * ln_idx + 2],
                            ALU.mult, ALU.add)

            # ---------------- LN1: x1 = LN(ywin) ----------------
            x1 = cp.tile([128, 4, WT], BF16, tag="x1")
            layer_norm(lambda et: ywin[:, et, :], 0, lambda et: x1[:, et, :], WT)

            # ---------------- attention: q/k/v projections ----------------
            q_sb = cp.tile([128, 4, TL], BF16, tag="q")
            k_sb = cp.tile([128, 4, WT], BF16, tag="k")
            for h in range(4):
                qp = pm.tile([128, TL], F32, tag="mm")
                for kk in range(4):
                    nc.tensor.matmul(qp, wqk_sb[:, kk, h * 128:(h + 1) * 128],
                                     x1[:, kk, WIN:WIN + TL],
                                     start=kk == 0, stop=kk == 3)
                nc.scalar.activation(q_sb[:, h, :], qp, AF.Copy)
                kp = pm.tile([128, TL], F32, tag="mm")
                kp2 = ps2.tile([128, 32], F32, tag="sc")
                for kk in range(4):
                    nc.tensor.matmul(kp, wqk_sb[:, 4 + kk, h * 128:(h + 1) * 128],
                                     x1[:, kk, 0:TL], start=kk == 0, stop=kk == 3)
                    nc.tensor.matmul(kp2, wqk_sb[:, 4 + kk, h * 128:(h + 1) * 128],
                                     x1[:, kk, TL:WT], start=kk == 0, stop=kk == 3)
                nc.scalar.activation(k_sb[:, h, 0:TL], kp, AF.Copy)
                nc.scalar.activation(k_sb[:, h, TL:WT], kp2, AF.Copy)

            # v token-major: [t' (part), d] chunks at x1-cols VSTARTS
            v_sb = cp.tile([128, 5, D], BF16, tag="v")
            for ci, (st, rows) in enumerate(VSTARTS):
                vp = pm.tile([128, D], F32, tag="mm")
                for kk in range(4):
                    nc.tensor.matmul(vp[0:rows, :],
                                     x1[:, kk, st:st + rows],
                                     wv_sb[:, kk, :],
                                     start=kk == 0, stop=kk == 3)
                nc.vector.tensor_copy(v_sb[0:rows, ci, :], vp[0:rows, :])

            # ---------------- banded softmax attention ----------------
            a2 = cp.tile([128, 4, TL], BF16, tag="a2")
            rec_sb = cp.tile([128, TL], F32, tag="rec")
            lnden = cp.tile([128, TL], F32, tag="lnden")
            for h in range(4):
                den = pb.tile([128, TL], F32, tag="cd")
                ao = pa.tile([128, TL], F32, tag="s12")
                for q0, qn, pieces, mcol in QBLOCKS:
                    qsl = slice(q0 - WIN, q0 - WIN + qn)   # q_sb index space
                    bsl = slice(q0 - WIN, q0 - WIN + qn)   # block cols in den/ao
                    for pi, (pst, plen) in enumerate(pieces):
                        sp = ps2.tile([128, 224], F32, tag="sc")
                        nc.tensor.matmul(sp[0:plen, 0:qn],
                                         k_sb[:, h, pst:pst + plen],
                                         q_sb[:, h, qsl],
                                         start=True, stop=True)
                        e = ep.tile([128, 224], BF16, tag="e")
                        nc.scalar.activation(e[0:plen, 0:qn], sp[0:plen, 0:qn],
                                             AF.Exp, scale=SCL)
                        nc.vector.tensor_mul(
                            e[0:plen, 0:qn], e[0:plen, 0:qn],
                            msk_sb[0:plen, mcol + pi * 224:mcol + pi * 224 + qn])
                        nc.tensor.matmul(den[:, bsl], ones_sb[0:plen, :],
                                         e[0:plen, 0:qn],
                                         start=pi == 0, stop=pi == len(pieces) - 1)
                        ci = VSTARTS.index((pst, plen))
                        nc.tensor.matmul(ao[:, bsl],
                                         v_sb[0:plen, ci, h * 128:(h + 1) * 128],
                                         e[0:plen, 0:qn],
                                         start=pi == 0, stop=pi == len(pieces) - 1)
                # 1/den via exp(-ln(den)) — stays in the ln/exp table
                nc.scalar.activation(lnden, den, AF.Ln)
                nc.scalar.activation(rec_sb, lnden, AF.Exp, scale=-1.0)
                nc.vector.tensor_mul(a2[:, h, :], ao, rec_sb)

            # ---------------- proj + residual, LN2 ----------------
            x2pre = cp.tile([128, 4, TL], BF16, tag="x2pre")
            for et in range(4):
                pp = pm.tile([128, TL], F32, tag="mm")
                for kk in range(4):
                    nc.tensor.matmul(pp, wp_sb[:, kk, et * 128:(et + 1) * 128],
                                     a2[:, kk, :], start=kk == 0, stop=kk == 3)
                nc.vector.tensor_add(x2pre[:, et, :],
                                     x1[:, et, WIN:WIN + TL], pp)
            x2 = cp.tile([128, 4, TL], BF16, tag="x2")
            layer_norm(lambda et: x2pre[:, et, :], 1,
                       lambda et: x2[:, et, :], TL)

            # ---------------- FFN (pre-LN) ----------------
            xf = cp.tile([128, 4, TL], BF16, tag="xf")
            layer_norm(lambda et: x2[:, et, :], 2, lambda et: xf[:, et, :], TL)

            hg = cp.tile([128, 16, TL], BF16, tag="hg")
            ops = [pm.tile([128, TL], F32, tag="mm", name="op0"),
                   pm.tile([128, TL], F32, tag="mm", name="op1"),
                   pb.tile([128, TL], F32, tag="cd", name="op2"),
                   pb.tile([128, TL], F32, tag="cd", name="op3")]
            for kk in range(16):
                hp = ps2.tile([128, TL], F32, tag="sc")
                for ki in range(4):
                    nc.tensor.matmul(hp, w1_sb[:, ki, kk * 128:(kk + 1) * 128],
                                     xf[:, ki, :], start=ki == 0, stop=ki == 3)
                nc.scalar.activation(hg[:, kk, :], hp, AF.Gelu,
                                     bias=b1_sb[:, kk:kk + 1])
                for et in range(4):
                    nc.tensor.matmul(ops[et], w2_sb[:, kk, et * 128:(et + 1) * 128],
                                     hg[:, kk, :],
                                     start=kk == 0, stop=kk == 15)
            x3 = cp.tile([128, 4, TL], F32, tag="x3")
            x3b = cp.tile([128, 4, TL], BF16, tag="x3b")
            for et in range(4):
                nc.vector.scalar_tensor_tensor(
                    x3[:, et, :], ops[et], b2_sb[:, et:et + 1],
                    x2[:, et, :], ALU.add, ALU.add)
                nc.scalar.activation(x3b[:, et, :], x3[:, et, :], AF.Copy)

            # ---------------- LN4 -> output ----------------
            outt = cp.tile([128, 4, TL], F32, tag="outt")
            layer_norm(lambda et: x3[:, et, :], 3, lambda et: outt[:, et, :], TL,
                       mm_in=lambda et: x3b[:, et, :])
            for et in range(4):
                nc.sync.dma_start(out_d[et, :, :], outt[:, et, :])

    nc.compile()
    return nc


def _host_inputs(x, W_in, W_state, glru_g, glru_b, Wq, Wk, Wv, Wp, attn_g,
                 attn_b, ffn_g, ffn_b, W1, b1, W2, b2, out_g, out_b):
    bf = ml_dtypes.bfloat16
    f32 = np.float32
    cb = lambda a: np.ascontiguousarray(np.asarray(a, dtype=f32)).astype(bf)
    W_in = np.asarray(W_in, f32)
    W_state = np.asarray(W_state, f32)
    wc_mat = W_state @ W_in[D:]              # cand = x @ wc_mat.T
    # lncol[p, et, 2*ln+(g|b)] : per-feature gamma/beta columns
    lncol = np.zeros((128, 4, 8), f32)
    for ln, (g, b) in enumerate([(glru_g, glru_b), (attn_g, attn_b),
                                 (ffn_g, ffn_b), (out_g, out_b)]):
        g = np.asarray(g, f32).reshape(4, 128)
        b = np.asarray(b, f32).reshape(4, 128)
        for et in range(4):
            lncol[:, et, 2 * ln] = g[et]
            lncol[:, et, 2 * ln + 1] = b[et]
    shared = {
        "wu": cb(W_in[:D].T), "wc": cb(wc_mat.T),
        "wq": cb(np.asarray(Wq, f32).T), "wk": cb(np.asarray(Wk, f32).T),
        "wv": cb(np.asarray(Wv, f32).T), "wp": cb(np.asarray(Wp, f32).T),
        "w1": cb(np.asarray(W1, f32).T), "w2": cb(np.asarray(W2, f32).T),
        "lncol": lncol,
        "b1c": np.ascontiguousarray(
            np.asarray(b1, f32).reshape(FFD // 128, 128).T),
        "b2c": np.ascontiguousarray(
            np.asarray(b2, f32).reshape(D // 128, 128).T),
    }
    in_maps = []
    for core in range(NCORES):
        b_, half = core // 2, core % 2
        h0 = half * TL
        m = dict(shared)
        m["xt"] = cb(np.asarray(x, f32)[b_].T)
        m["cw"] = np.broadcast_to(
            np.array([1.0 - half, float(half)], f32), (128, 2)).copy()
        # masks per (block, piece): band |kc-qc|<=16 and true k-token in range
        msk = np.zeros((128, MSKW), f32)
        for q0, qn, pieces, mcol in QBLOCKS:
            for pi, (pst, plen) in enumerate(pieces):
                r = np.arange(plen)
                c = np.arange(qn)
                kc = pst + r
                qc = q0 + c
                tk = h0 - WIN + kc
                band = (np.abs(kc[:, None] - qc[None, :]) <= WIN) \
                    & (tk[:, None] >= 0) & (tk[:, None] < T)
                msk[0:plen, mcol + pi * 224:mcol + pi * 224 + qn] = band
        m["msk"] = msk.astype(bf)
        in_maps.append(m)
    return in_maps


def kernel(**inputs):
    if "nc" not in _CACHE:
        _CACHE["nc"] = _build_nc()
    nc = _CACHE["nc"]
    in_maps = _host_inputs(**inputs)
    res = run_bass_kernel_spmd(nc, in_maps, core_ids=list(range(NCORES)),
                               **_CACHE.get("run_kwargs", {}))
    _CACHE["last_result"] = res
    out = np.empty((B, T, D), np.float32)
    for core in range(NCORES):
        b_, half = core // 2, core % 2
        o = np.asarray(res.results[core]["outp"], dtype=np.float32)  # [4,128,TL]
        out[b_, half * TL:(half + 1) * TL, :] = o.reshape(D, TL).T
    return out
